# revision 11
# baseline (speedup 1.0000x reference)
"""CapsuleLayer kernel for 8 Trainium2 NeuronCores.

Math: with b0 = 0, softmax(b0, axis=1) is exactly uniform (1/N), so
outputs[b,i,k] = squash_k((1/N) * sum_j inputs_hat[b,j,k]) independent of i.
The b-update keeps b constant along axis 1, so softmax stays exactly uniform
and all routing iterations return the same outputs. Hence:

    Wsum[m,k] = sum_j W[j,m,k]
    v[b,k]    = (1/N) * (inputs @ Wsum)[b,k]
    out[b,i,k] = squash_k(v)[b,k]          (broadcast over i)

Kernel 1 (m-sharded): core c reduces W[:, 32c:32c+32, :] over j -> Wsum rows.
Kernel 2 (batch-sharded): core c computes squash((inputs_c @ Wsum)/N) and
broadcast-writes its [64, 256, 256] output slice.
"""

import numpy as np

import concourse.bass as bass
import concourse.mybir as mybir
import concourse.tile as tile
from concourse.ap import AP
from concourse.bass_utils import run_bass_kernel_spmd

F32 = mybir.dt.float32

B, N = 512, 256
NCORES = 8
BPC = B // NCORES  # 64 batch rows per core (kernel 2)
MPC = N // NCORES  # 32 m rows per core (kernel 1)
REPS = 16          # copies of the 256-wide s row kept per partition
EPS = 1e-7

_CACHE = {}


def _fix_multiwait(nc, maxw=1):
    """This walrus build rejects instructions carrying more than one sync
    wait ("Too many sync wait commands"). Hoist extra waits into standalone
    single-wait EventSemaphore instructions on the same engine, placed
    immediately before the offender."""
    ctr = 0
    for fn in nc.m.functions:
        for bb in fn.blocks:
            out = []
            for ins in bb.instructions:
                si = ins.sync_info
                if si is not None and len(si.on_wait) > maxw:
                    waits = list(si.on_wait)
                    for w in waits[:-maxw]:
                        ctr += 1
                        ev = mybir.InstEventSemaphore(
                            name=f"mwsplit-{ctr}",
                            engine=ins.engine,
                            ins=[],
                            outs=[],
                            sync_info=mybir.SyncInfo(on_wait=[w], on_update=[]),
                        )
                        nc.register_instruction(ev, overwrite=True)
                        out.append(ev)
                    si.on_wait = waits[-maxw:]
                    ins.sync_info = si
                out.append(ins)
            bb.instructions[:] = out
    return nc

# Exec times (ns) of the last traced run, for test harnesses.
LAST_EXEC_NS = {"k1": None, "k2": None}


def _build_k1():
    """Reduce the per-core W slice over j.

    Input  w_in [256 (j), 8193]: cols 0..8191 = W[j, mslice, :] flat,
           col 8192 = 1.0 (the matmul ones-column rides the same DMA so
           every instruction needs at most ONE semaphore wait — walrus
           rejects matmuls with 2+ waits).
    Output wsum_part [1, 8192]  (= Wsum[mslice, :] flat)
    """
    nc = bass.Bass()
    FREE = MPC * N    # 8192
    ROW = FREE + 1    # 8193 (data + ones column)
    MMF = 512         # moving free dim per matmul

    w = nc.dram_tensor("w_in", [N, ROW], F32, kind="ExternalInput")
    wsum = nc.dram_tensor("wsum_part", [1, FREE], F32, kind="ExternalOutput")

    with tile.TileContext(nc) as tc:
        with (
            tc.tile_pool(name="singles", bufs=1) as singles,
            tc.tile_pool(name="psum", bufs=8, space="PSUM") as psum_pool,
        ):
            acc = singles.tile([1, FREE], F32)

            # One 8.4 MB DMA (single semaphore lane): T[p, jc*ROW + f]
            # = w_in[jc*128 + p, f].
            t = singles.tile([128, 2 * ROW], F32)
            src = AP(tensor=w, offset=0, ap=[[ROW, 128], [128 * ROW, 2], [1, ROW]])
            dst = t[:].rearrange("p (jc f) -> p jc f", jc=2)
            nc.sync.dma_start(out=dst, in_=src)

            ones = t[:, FREE:FREE + 1]  # [128, 1] of 1.0

            for g in range(FREE // MMF):  # 16 groups of 512 outputs
                ps = psum_pool.tile([1, MMF], F32)
                nc.tensor.matmul(
                    ps[:], lhsT=ones, rhs=t[:, g * MMF:(g + 1) * MMF],
                    start=True, stop=False,
                )
                nc.tensor.matmul(
                    ps[:], lhsT=ones, rhs=t[:, ROW + g * MMF:ROW + (g + 1) * MMF],
                    start=False, stop=True,
                )
                nc.vector.tensor_copy(
                    out=acc[0:1, g * MMF:(g + 1) * MMF], in_=ps[:]
                )

            nc.sync.dma_start(out=wsum[:], in_=acc[:])
    return nc


def _build_k2():
    """Per-core: u = inputs_c @ Wsum, s = squash(u/N), broadcast-write output.

    Inputs  xt   [256 (m), 64 (b)]   (= inputs_c.T)
            wsum [256 (m), 256 (k)]
    Output  out  [BPC*N*N] flat = out[b, i, k] with value s[b, k].

    PSUM partition q = 2*b + ihalf (interleaved duplicate of b), so the flat
    output address q*(N*128) + g*(16*N) + t is affine per DMA g.
    """
    nc = bass.Bass()
    xt = nc.dram_tensor("xt", [N, BPC], F32, kind="ExternalInput")
    ws = nc.dram_tensor("wsum", [N, N], F32, kind="ExternalInput")
    out = nc.dram_tensor("out", [BPC * N * N], F32, kind="ExternalOutput")

    SREP_W = REPS * N          # 4096 elements per partition
    NDMA = (N // 2) // REPS    # 8 output DMAs, one per group of 16 i-rows

    with tile.TileContext(nc) as tc:
        with (
            tc.tile_pool(name="sb", bufs=1) as sb,
            tc.tile_pool(name="psum", bufs=1, space="PSUM") as psum_pool,
        ):
            # Load inputs_c.T halves and Wsum halves (contraction dim m on
            # partitions).
            xt0 = sb.tile([128, BPC], F32)
            nc.sync.dma_start(out=xt0[:], in_=xt[0:128, :])
            xt1 = sb.tile([128, BPC], F32)
            nc.sync.dma_start(out=xt1[:], in_=xt[128:256, :])
            ws0 = sb.tile([128, N], F32)
            nc.sync.dma_start(out=ws0[:], in_=ws[0:128, :])
            ws1 = sb.tile([128, N], F32)
            nc.sync.dma_start(out=ws1[:], in_=ws[128:256, :])

            # Duplicate b columns interleaved: xd[:, 2b + d] = xt[:, b].
            # Both matmul operands are produced by DVE so each matmul needs
            # only ONE semaphore wait (walrus limit).
            xd0 = sb.tile([128, 2 * BPC], F32)
            xd1 = sb.tile([128, 2 * BPC], F32)
            for xd, xsrc in ((xd0, xt0), (xd1, xt1)):
                pairs = xd[:].rearrange("p (b two) -> p b two", two=2)
                nc.vector.tensor_copy(out=pairs[:, :, 0], in_=xsrc[:])
                nc.vector.tensor_copy(out=pairs[:, :, 1], in_=xsrc[:])
            wsv0 = sb.tile([128, N], F32)
            nc.vector.tensor_copy(out=wsv0[:], in_=ws0[:])
            wsv1 = sb.tile([128, N], F32)
            nc.vector.tensor_copy(out=wsv1[:], in_=ws1[:])

            # u[q, k] = sum_m inputs_c[q//2, m] * Wsum[m, k]
            u = psum_pool.tile([128, N], F32)
            nc.tensor.matmul(u[:], lhsT=xd0[:], rhs=wsv0[:], start=True, stop=False)
            nc.tensor.matmul(u[:], lhsT=xd1[:], rhs=wsv1[:], start=False, stop=True)

            # squash: v = u/N; s2 = sum_k v^2; s = v * s2/(1+s2)/sqrt(s2+eps)
            #       = u * factor,  factor = s2/(1+s2)/sqrt(s2+eps)/N
            sq = sb.tile([128, N], F32)
            s2 = sb.tile([128, 1], F32)
            nc.scalar.activation(
                out=sq[:], in_=u[:], func=mybir.ActivationFunctionType.Square,
                scale=1.0 / N, accum_out=s2[:],
            )
            eps_t = sb.tile([128, 1], F32)
            nc.vector.memset(eps_t[:], EPS)
            r = sb.tile([128, 1], F32)
            nc.scalar.activation(
                out=r[:], in_=s2[:], func=mybir.ActivationFunctionType.Sqrt,
                bias=eps_t[:],
            )
            onep = sb.tile([128, 1], F32)
            nc.vector.tensor_scalar_add(onep[:], s2[:], 1.0)
            den = sb.tile([128, 1], F32)
            nc.vector.tensor_mul(den[:], onep[:], r[:])
            rec = sb.tile([128, 1], F32)
            nc.vector.reciprocal(rec[:], den[:])
            fac = sb.tile([128, 1], F32)
            nc.vector.tensor_mul(fac[:], s2[:], rec[:])
            nc.vector.tensor_scalar_mul(fac[:], fac[:], 1.0 / N)

            # s_rep[q, r*N + k] = s[q//2, k] for r in range(REPS)
            s_rep = sb.tile([128, SREP_W], F32)
            nc.vector.tensor_scalar(
                s_rep[:, 0:N], u[:], fac[:], None, mybir.AluOpType.mult
            )
            width = N
            while width < SREP_W:
                w2 = min(width, SREP_W - width)
                nc.vector.tensor_copy(
                    out=s_rep[:, width:width + w2], in_=s_rep[:, 0:w2]
                )
                width += w2

            # DMA g writes out[q*32768 + g*4096 + t] = s_rep[q, t]:
            # b = q//2, i = (q%2)*128 + g*16 + t//256, k = t%256.
            for g in range(NDMA):
                dst = AP(
                    tensor=out,
                    offset=g * SREP_W,
                    ap=[[128 * N, 128], [1, SREP_W]],
                )
                nc.sync.dma_start(out=dst, in_=s_rep[:])
    return nc


def _run(nc, in_maps, core_ids, trace):
    if trace:
        try:
            return run_bass_kernel_spmd(nc, in_maps, core_ids, trace=True)
        except Exception as e:  # noqa: BLE001
            print(f"kernel: trace run failed ({e}); rerunning without trace")
    return run_bass_kernel_spmd(nc, in_maps, core_ids, trace=False)


def _get(name):
    if name not in _CACHE:
        _CACHE[name] = _fix_multiwait(_build_k1() if name == "k1" else _build_k2())
    return _CACHE[name]


def kernel(inputs: np.ndarray, W: np.ndarray, trace: bool = False) -> np.ndarray:
    inputs = np.ascontiguousarray(inputs, dtype=np.float32)
    W = np.ascontiguousarray(W, dtype=np.float32)
    core_ids = list(range(NCORES))

    # ---- kernel 1: Wsum rows, m-sharded ----
    k1 = _get("k1")
    in_maps1 = []
    for c in core_ids:
        w_in = np.empty((N, MPC * N + 1), dtype=np.float32)
        w_in[:, : MPC * N] = W[:, c * MPC:(c + 1) * MPC, :].reshape(N, MPC * N)
        w_in[:, MPC * N] = 1.0
        in_maps1.append({"w_in": w_in})
    res1 = _run(k1, in_maps1, core_ids, trace)
    LAST_EXEC_NS["k1"] = res1.exec_time_ns
    wsum = np.concatenate(
        [res1.results[c]["wsum_part"].reshape(MPC, N) for c in core_ids], axis=0
    )  # [256, 256]

    # ---- kernel 2: squash + broadcast write, batch-sharded ----
    k2 = _get("k2")
    xt_full = np.ascontiguousarray(inputs.T)  # [256, 512]
    in_maps2 = [
        {
            "xt": np.ascontiguousarray(xt_full[:, c * BPC:(c + 1) * BPC]),
            "wsum": wsum,
        }
        for c in core_ids
    ]
    res2 = _run(k2, in_maps2, core_ids, trace)
    LAST_EXEC_NS["k2"] = res2.exec_time_ns
    out = np.concatenate(
        [res2.results[c]["out"].reshape(BPC, N, N) for c in core_ids], axis=0
    )
    return out


# revision 15
# speedup vs baseline: 1.1326x; 1.1326x over previous
"""CapsuleLayer kernel for 8 Trainium2 NeuronCores.

Math: with b0 = 0, softmax(b0, axis=1) is exactly uniform (1/N), so
outputs[b,i,k] = squash_k((1/N) * sum_j inputs_hat[b,j,k]) independent of i.
The b-update keeps b constant along axis 1, so softmax stays exactly uniform
and all routing iterations return the same outputs. Hence:

    Wsum[m,k] = sum_j W[j,m,k]
    v[b,k]    = (1/N) * (inputs @ Wsum)[b,k]
    out[b,i,k] = squash_k(v)[b,k]          (broadcast over i)

Kernel 1 (m-sharded): core c reduces W[:, 32c:32c+32, :] over j -> Wsum rows.
Kernel 2 (batch-sharded): core c computes squash((inputs_c @ Wsum)/N) and
broadcast-writes its [64, 256, 256] output slice.
"""

import numpy as np

import concourse.bass as bass
import concourse.mybir as mybir
import concourse.tile as tile
from concourse.ap import AP
from concourse.bass_utils import run_bass_kernel_spmd

F32 = mybir.dt.float32

B, N = 512, 256
NCORES = 8
BPC = B // NCORES  # 64 batch rows per core (kernel 2)
MPC = N // NCORES  # 32 m rows per core (kernel 1)
REPS = 16          # copies of the 256-wide s row kept per partition
EPS = 1e-7

_CACHE = {}


def _fix_multiwait(nc, maxw=1):
    """This walrus build rejects instructions carrying more than one sync
    wait ("Too many sync wait commands"). Hoist extra waits into standalone
    single-wait EventSemaphore instructions on the same engine, placed
    immediately before the offender."""
    ctr = 0
    for fn in nc.m.functions:
        for bb in fn.blocks:
            out = []
            for ins in bb.instructions:
                si = ins.sync_info
                if si is not None and len(si.on_wait) > maxw:
                    waits = list(si.on_wait)
                    for w in waits[:-maxw]:
                        ctr += 1
                        ev = mybir.InstEventSemaphore(
                            name=f"mwsplit-{ctr}",
                            engine=ins.engine,
                            ins=[],
                            outs=[],
                            sync_info=mybir.SyncInfo(on_wait=[w], on_update=[]),
                        )
                        nc.register_instruction(ev, overwrite=True)
                        out.append(ev)
                    si.on_wait = waits[-maxw:]
                    ins.sync_info = si
                out.append(ins)
            bb.instructions[:] = out
    return nc

# Exec times (ns) of the last traced run, for test harnesses.
LAST_EXEC_NS = {"k1": None, "k2": None}


def _build_k1():
    """Reduce the per-core W slice over j.

    Input  w_in [256 (j), 8192 (m_local*256 + k)]  (= W[:, mslice, :] flat)
    Output wsum_part [1, 8192]  (= Wsum[mslice, :] flat)

    Pipeline per 2048-wide chunk: DMA both j-halves, DVE-add them (j 256->128),
    then PE ones-matmuls reduce the 128 partitions; DVE copies PSUM->acc.
    The _fix_multiwait post-pass legalizes any multi-wait instruction, so
    loads/compute overlap freely.
    """
    nc = bass.Bass()
    FREE = MPC * N    # 8192
    CHUNK = 2048
    MMF = 512         # moving free dim per matmul

    w = nc.dram_tensor("w_in", [N, FREE], F32, kind="ExternalInput")
    wsum = nc.dram_tensor("wsum_part", [1, FREE], F32, kind="ExternalOutput")

    with tile.TileContext(nc) as tc:
        with (
            tc.tile_pool(name="singles", bufs=1) as singles,
            tc.tile_pool(name="wpool", bufs=2) as wpool,
            tc.tile_pool(name="psum", bufs=8, space="PSUM") as psum_pool,
        ):
            ones = singles.tile([128, 1], F32)
            nc.vector.memset(ones[:], 1.0)
            acc = singles.tile([1, FREE], F32)

            for c in range(FREE // CHUNK):  # 4 chunks of 2048
                sl = slice(c * CHUNK, (c + 1) * CHUNK)
                ta = wpool.tile([128, CHUNK], F32, tag="ta")
                nc.sync.dma_start(out=ta[:], in_=w[0:128, sl])
                tb = wpool.tile([128, CHUNK], F32, tag="tb")
                nc.sync.dma_start(out=tb[:], in_=w[128:256, sl])
                ts = wpool.tile([128, CHUNK], F32, tag="ts")
                nc.vector.tensor_add(ts[:], ta[:], tb[:])
                for g in range(CHUNK // MMF):  # 4 matmul groups of 512
                    ps = psum_pool.tile([1, MMF], F32)
                    nc.tensor.matmul(
                        ps[:], lhsT=ones[:], rhs=ts[:, g * MMF:(g + 1) * MMF],
                        start=True, stop=True,
                    )
                    osl = slice(c * CHUNK + g * MMF, c * CHUNK + (g + 1) * MMF)
                    nc.vector.tensor_copy(out=acc[0:1, osl], in_=ps[:])

            nc.sync.dma_start(out=wsum[:], in_=acc[:])
    return nc


def _build_k2():
    """Per-core: u = inputs_c @ Wsum, s = squash(u/N), broadcast-write output.

    Inputs  xt   [256 (m), 64 (b)]   (= inputs_c.T)
            wsum [256 (m), 256 (k)]
    Output  out  [BPC*N*N] flat = out[b, i, k] with value s[b, k].

    PSUM partition q = 2*b + ihalf (interleaved duplicate of b), so the flat
    output address q*(N*128) + g*(16*N) + t is affine per DMA g.
    """
    nc = bass.Bass()
    xt = nc.dram_tensor("xt", [N, BPC], F32, kind="ExternalInput")
    ws = nc.dram_tensor("wsum", [N, N], F32, kind="ExternalInput")
    out = nc.dram_tensor("out", [BPC * N * N], F32, kind="ExternalOutput")

    SREP_W = REPS * N          # 4096 elements per partition
    NDMA = (N // 2) // REPS    # 8 output DMAs, one per group of 16 i-rows

    with tile.TileContext(nc) as tc:
        with (
            tc.tile_pool(name="sb", bufs=1) as sb,
            tc.tile_pool(name="psum", bufs=1, space="PSUM") as psum_pool,
        ):
            # Load inputs_c.T halves and Wsum halves (contraction dim m on
            # partitions).
            xt0 = sb.tile([128, BPC], F32)
            nc.sync.dma_start(out=xt0[:], in_=xt[0:128, :])
            xt1 = sb.tile([128, BPC], F32)
            nc.sync.dma_start(out=xt1[:], in_=xt[128:256, :])
            ws0 = sb.tile([128, N], F32)
            nc.scalar.dma_start(out=ws0[:], in_=ws[0:128, :])
            ws1 = sb.tile([128, N], F32)
            nc.scalar.dma_start(out=ws1[:], in_=ws[128:256, :])

            # Duplicate b columns interleaved: xd[:, 2b + d] = xt[:, b].
            xd0 = sb.tile([128, 2 * BPC], F32)
            xd1 = sb.tile([128, 2 * BPC], F32)
            for xd, xsrc in ((xd0, xt0), (xd1, xt1)):
                pairs = xd[:].rearrange("p (b two) -> p b two", two=2)
                nc.vector.tensor_copy(out=pairs[:, :, 0], in_=xsrc[:])
                nc.vector.tensor_copy(out=pairs[:, :, 1], in_=xsrc[:])

            # u[q, k] = sum_m inputs_c[q//2, m] * Wsum[m, k]
            u = psum_pool.tile([128, N], F32)
            nc.tensor.matmul(u[:], lhsT=xd0[:], rhs=ws0[:], start=True, stop=False)
            nc.tensor.matmul(u[:], lhsT=xd1[:], rhs=ws1[:], start=False, stop=True)

            # squash: v = u/N; s2 = sum_k v^2; s = v * s2/(1+s2)/sqrt(s2+eps)
            #       = u * factor,  factor = s2/(1+s2)/sqrt(s2+eps)/N
            sq = sb.tile([128, N], F32)
            s2 = sb.tile([128, 1], F32)
            nc.scalar.activation(
                out=sq[:], in_=u[:], func=mybir.ActivationFunctionType.Square,
                scale=1.0 / N, accum_out=s2[:],
            )
            eps_t = sb.tile([128, 1], F32)
            nc.vector.memset(eps_t[:], EPS)
            r = sb.tile([128, 1], F32)
            nc.scalar.activation(
                out=r[:], in_=s2[:], func=mybir.ActivationFunctionType.Sqrt,
                bias=eps_t[:],
            )
            den = sb.tile([128, 1], F32)
            nc.vector.scalar_tensor_tensor(
                den[:], s2[:], 1.0, r[:],
                op0=mybir.AluOpType.add, op1=mybir.AluOpType.mult,
            )
            rec = sb.tile([128, 1], F32)
            nc.vector.reciprocal(rec[:], den[:])
            fac = sb.tile([128, 1], F32)
            nc.vector.scalar_tensor_tensor(
                fac[:], s2[:], 1.0 / N, rec[:],
                op0=mybir.AluOpType.mult, op1=mybir.AluOpType.mult,
            )

            # s_rep[q, r*N + k] = s[q//2, k] for r in range(REPS)
            s_rep = sb.tile([128, SREP_W], F32)
            nc.vector.tensor_scalar(
                s_rep[:, 0:N], u[:], fac[:], None, mybir.AluOpType.mult
            )
            width = N
            while width < SREP_W:
                w2 = min(width, SREP_W - width)
                nc.vector.tensor_copy(
                    out=s_rep[:, width:width + w2], in_=s_rep[:, 0:w2]
                )
                width += w2

            # DMA g writes out[q*32768 + g*4096 + t] = s_rep[q, t]:
            # b = q//2, i = (q%2)*128 + g*16 + t//256, k = t%256.
            for g in range(NDMA):
                dst = AP(
                    tensor=out,
                    offset=g * SREP_W,
                    ap=[[128 * N, 128], [1, SREP_W]],
                )
                eng = nc.sync if g % 2 == 0 else nc.scalar
                eng.dma_start(out=dst, in_=s_rep[:])
    return nc


def _run(nc, in_maps, core_ids, trace):
    if trace:
        try:
            return run_bass_kernel_spmd(nc, in_maps, core_ids, trace=True)
        except Exception as e:  # noqa: BLE001
            print(f"kernel: trace run failed ({e}); rerunning without trace")
    return run_bass_kernel_spmd(nc, in_maps, core_ids, trace=False)


def _get(name):
    if name not in _CACHE:
        _CACHE[name] = _fix_multiwait(_build_k1() if name == "k1" else _build_k2())
    return _CACHE[name]


def kernel(inputs: np.ndarray, W: np.ndarray, trace: bool = False) -> np.ndarray:
    inputs = np.ascontiguousarray(inputs, dtype=np.float32)
    W = np.ascontiguousarray(W, dtype=np.float32)
    core_ids = list(range(NCORES))

    # ---- kernel 1: Wsum rows, m-sharded ----
    k1 = _get("k1")
    in_maps1 = [
        {
            "w_in": np.ascontiguousarray(
                W[:, c * MPC:(c + 1) * MPC, :]
            ).reshape(N, MPC * N)
        }
        for c in core_ids
    ]
    res1 = _run(k1, in_maps1, core_ids, trace)
    LAST_EXEC_NS["k1"] = res1.exec_time_ns
    wsum = np.concatenate(
        [res1.results[c]["wsum_part"].reshape(MPC, N) for c in core_ids], axis=0
    )  # [256, 256]

    # ---- kernel 2: squash + broadcast write, batch-sharded ----
    k2 = _get("k2")
    xt_full = np.ascontiguousarray(inputs.T)  # [256, 512]
    in_maps2 = [
        {
            "xt": np.ascontiguousarray(xt_full[:, c * BPC:(c + 1) * BPC]),
            "wsum": wsum,
        }
        for c in core_ids
    ]
    res2 = _run(k2, in_maps2, core_ids, trace)
    LAST_EXEC_NS["k2"] = res2.exec_time_ns
    out = np.concatenate(
        [res2.results[c]["out"].reshape(BPC, N, N) for c in core_ids], axis=0
    )
    return out
